# revision 39
# baseline (speedup 1.0000x reference)
"""Trainium2 Bass kernel for 16-head causal RoPE attention (B=1, L=4096, D=1024).

Distribution: tensor-parallel over heads - each of the 8 cores owns 2 heads
(128 q/k/v dims) and computes a partial output projection; the host sums the
8 partial [1024, 4096] bf16 outputs and transposes back to [1, 4096, 1024].

Per-core dataflow (bf16 compute, f32 PSUM accumulation):
  x chunks [128, 8, 512] bf16 -> q/k/vT [128, 512] via PE matmuls
  RoPE via a PE permutation matmul + DVE muls/add -> qro/kro bf16
  vT transposed per 128-chunk on PE into vno [kv, 64+1(ones)+64+1] lhsT
  scores^T [128 kv, 512 q] per head, both heads ROW-PACKED on the PE array
  (tile_position (0,0)/(64,0), K=64 each) into one [128, 2, 512] psum tile
  exp on ACT (no max subtraction; scores are O(5)); causal handled by
  sub-ranging the diagonal blocks + one in-place [128, 2, 128] triangle mask
  attn@v per head with a fused ones-column producing the softmax denominator
  in psum row 64; normalization via DRAM-bounce partition-broadcast +
  fast-approx reciprocal
  final projection woT.T @ outT -> yT [1024, L] bf16 partial, streamed out
"""

import numpy as np

N_HEAD = 16
HEAD_DIM = 64
HIDDEN = 1024
N_CORES = 8
ROPE_BASE = 10000.0

_CACHE = {}


def _build(L):
    import concourse.bass as bass
    import concourse.tile as tile
    import concourse.mybir as mybir
    from concourse import bacc
    from concourse.masks import make_identity

    F32 = mybir.dt.float32
    BF16 = mybir.dt.bfloat16
    Exp = mybir.ActivationFunctionType.Exp

    LC = L // 512          # number of 512-wide q chunks
    KVC = L // 128         # number of 128-wide kv chunks
    HC = HIDDEN // 128     # hidden contraction chunks

    nc = bacc.Bacc("TRN2", target_bir_lowering=False, debug=False,
                   num_devices=N_CORES)

    xt_d = nc.dram_tensor("xt", [LC, 128, HC, 512], BF16, kind="ExternalInput")
    wqT_d = nc.dram_tensor("wqT", [128, HC, 128], BF16, kind="ExternalInput")
    wkT_d = nc.dram_tensor("wkT", [128, HC, 128], BF16, kind="ExternalInput")
    wvT_d = nc.dram_tensor("wvT", [128, HC, 128], BF16, kind="ExternalInput")
    woT_d = nc.dram_tensor("woT", [128, HIDDEN], BF16, kind="ExternalInput")
    cs_d = nc.dram_tensor("cs", [LC, 128, 2, 512], BF16, kind="ExternalInput")
    tri_d = nc.dram_tensor("tri", [128, 2, 128], BF16, kind="ExternalInput")
    osel_d = nc.dram_tensor("osel", [1, 2, 128], F32, kind="ExternalInput")
    pmat_d = nc.dram_tensor("pmat", [128, 128], BF16, kind="ExternalInput")
    yT_d = nc.dram_tensor("yT", [HIDDEN, L], BF16, kind="ExternalOutput")

    with tile.TileContext(nc) as tc:
        with tc.tile_pool(name="big", bufs=1) as big, \
             tc.tile_pool(name="w_p", bufs=1) as w_p, \
             tc.tile_pool(name="sm2", bufs=2) as sm2, \
             tc.tile_pool(name="xt_p", bufs=3) as xt_p, \
             tc.tile_pool(name="att_p", bufs=6) as att_p, \
             tc.tile_pool(name="ps_acc", bufs=2, space="PSUM") as ps_acc, \
             tc.tile_pool(name="ps_st", bufs=2, space="PSUM") as ps_st, \
             tc.tile_pool(name="ps_av", bufs=2, space="PSUM") as ps_av:

            # ---- constants / weights ----
            F32R = mybir.dt.float32r
            wq_sb = w_p.tile([128, HC, 128], BF16, tag="wq")
            wk_sb = w_p.tile([128, HC, 128], BF16, tag="wk")
            wv_sb = w_p.tile([128, HC, 128], BF16, tag="wv")
            wo_sb = w_p.tile([128, HIDDEN], BF16, tag="wo")
            tri_sb = w_p.tile([128, 2, 128], BF16, tag="tri")
            pmat_sb = w_p.tile([128, 128], BF16, tag="pmat")
            ident = w_p.tile([128, 128], BF16, tag="ident")
            make_identity(nc, ident)
            # per-head selector rows for the K=1 denominator-broadcast matmul
            ones_sel = w_p.tile([1, 2, 128], F32R, tag="ones_sel")
            nc.sync.dma_start(out=ones_sel, in_=osel_d.ap().bitcast(F32R))

            # v_nat with interleaved ones columns: [kv_in_chunk, kvc, 130]
            # cols 0:64 head0 dims, 64 = 1.0, 65:129 head1 dims, 129 = 1.0
            vno = big.tile([128, KVC, 130], BF16, tag="vno")
            nc.vector.memset(vno[:, :, 64:65], 1.0)
            nc.vector.memset(vno[:, :, 129:130], 1.0)

            qro = big.tile([128, L], BF16, tag="qro")
            kro = big.tile([128, L], BF16, tag="kro")

            prefetched = {}

            def prefetch(n):
                if n >= LC:
                    return
                xt = xt_p.tile([128, HC, 512], BF16, tag="xt")
                nc.sync.dma_start(out=xt, in_=xt_d.ap()[n])
                cs = xt_p.tile([128, 2, 512], BF16, tag="cs")
                nc.sync.dma_start(out=cs, in_=cs_d.ap()[n])
                prefetched[n] = (xt, cs)

            def phase1(n):
                ns = slice(n * 512, (n + 1) * 512)
                xt, cs = prefetched.pop(n)

                def accum(w_sb):
                    ps = ps_acc.tile([128, 512], F32, tag="acc")
                    for k in range(HC):
                        nc.tensor.matmul(ps, w_sb[:, k, :], xt[:, k, :],
                                         start=(k == 0), stop=(k == HC - 1))
                    t = sm2.tile([128, 512], BF16, tag="proj")
                    nc.vector.tensor_copy(t, ps)
                    return t

                def rope(t_sb, ro):
                    sw = ps_acc.tile([128, 512], F32, tag="acc")
                    nc.tensor.matmul(sw, pmat_sb, t_sb)
                    t1 = sm2.tile([128, 512], BF16, tag="t1")
                    nc.vector.tensor_mul(t1, t_sb, cs[:, 0, :])
                    t2 = sm2.tile([128, 512], BF16, tag="t2")
                    nc.vector.tensor_mul(t2, sw, cs[:, 1, :])
                    nc.vector.tensor_add(ro[:, ns], t1, t2)

                qt = accum(wq_sb)
                rope(qt, qro)
                kt = accum(wk_sb)
                rope(kt, kro)
                vt = accum(wv_sb)
                # v transpose into vno: 4 PE transposes into one psum bank,
                # then a single strided copy into the interleaved layout
                tr = ps_acc.tile([128, 4, 128], BF16, tag="acc")
                for j in range(4):
                    nc.tensor.transpose(tr[:, j, :],
                                        vt[:, j * 128:(j + 1) * 128], ident)
                trv = tr.rearrange("p j (h d) -> p j h d", h=2)
                dst = vno[:, n * 4:(n + 1) * 4, :] \
                    .rearrange("p j (h d) -> p j h d", h=2, d=65)[:, :, :, 0:64]
                nc.vector.tensor_copy(dst, trv)

            def attention_body(qc):
                qs = slice(qc * 512, (qc + 1) * 512)
                n_kc = 4 * (qc + 1)          # kv chunks (128 each), causal
                av0 = ps_av.tile([65, 512], F32, tag="av")
                av1 = ps_av.tile([65, 512], F32, tag="av")
                avs = [av0, av1]

                for kc in range(n_kc):
                    j = kc - 4 * qc          # >= 0 on the diagonal band
                    lo = max(0, j * 128)     # first valid q col in this chunk
                    ks = slice(kc * 128, (kc + 1) * 128)
                    stp = ps_st.tile([128, 2, 512], F32, tag="st")
                    for h in range(2):
                        hs = h * 64
                        nc.tensor.matmul(
                            stp[:, h, lo:],
                            kro[hs:hs + 64, ks],
                            qro[hs:hs + 64, qc * 512 + lo:(qc + 1) * 512],
                            tile_position=(hs, 0))
                    att = att_p.tile([128, 2, 512], BF16, tag="att")
                    nc.scalar.activation(att[:, :, lo:], stp[:, :, lo:], Exp)
                    if j >= 0:  # diagonal block: in-place triangle mask
                        # on GpSimd so it can't queue behind DVE bursts
                        nc.gpsimd.tensor_mul(
                            att[:, :, lo:lo + 128], att[:, :, lo:lo + 128],
                            tri_sb)
                    for h in range(2):
                        nc.tensor.matmul(
                            avs[h][:, lo:], vno[:, kc, h * 65:h * 65 + 65],
                            att[:, h, lo:],
                            start=(kc == 0), stop=(kc == n_kc - 1))

                # extract per-head outputs + reciprocal denominators
                # (frees avs psum)
                outT_un = sm2.tile([128, 512], BF16, tag="outT_un")
                rdens = []
                for h in range(2):
                    nc.vector.tensor_copy(outT_un[h * 64:(h + 1) * 64, :],
                                          avs[h][0:64, :])
                    den = sm2.tile([1, 512], F32, tag=f"den{h}")
                    nc.vector.tensor_copy(den, avs[h][64:65, :])
                    rden = sm2.tile([1, 512], F32, tag=f"rden{h}")
                    nc.vector.reciprocal_approx_fast(out=rden, in_=den)
                    rden_r = sm2.tile([1, 512], F32R, tag=f"rdenr{h}")
                    nc.vector.tensor_copy(rden_r, rden)
                    rdens.append(rden_r)
                return outT_un, rdens

            def attention_tail(qc, outT_un, rdens, last=False):
                # deferred normalization + projection: emitted after the NEXT
                # attention body so its psum/DVE serial chain does not block
                # the PE queue. 1/den is partition-broadcast by a K=1 matmul.
                qs = slice(qc * 512, (qc + 1) * 512)
                bci = ps_acc.tile([128, 512], F32, tag="acc")
                for h in range(2):
                    nc.tensor.matmul(bci, ones_sel[:, h, :], rdens[h],
                                     start=(h == 0), stop=(h == 1))
                outT = sm2.tile([128, 512], BF16, tag="outT")
                nc.vector.tensor_mul(outT, outT_un, bci)

                # final projection for this q chunk
                for e in range(HC):
                    ps_y = ps_acc.tile([128, 512], F32, tag="acc")
                    nc.tensor.matmul(ps_y, wo_sb[:, e * 128:(e + 1) * 128],
                                     outT)
                    y_sb = sm2.tile([128, 512], BF16, tag="y")
                    if last and e % 2 == 1:
                        # final chunk: ACT is idle, split drains across engines
                        nc.scalar.copy(y_sb, ps_y)
                    else:
                        nc.vector.tensor_copy(y_sb, ps_y)
                    nc.sync.dma_start(
                        out=yT_d.ap()[e * 128:(e + 1) * 128, qs], in_=y_sb)

            prefetch(0)
            nc.sync.dma_start(out=wq_sb, in_=wqT_d.ap())
            nc.sync.dma_start(out=wk_sb, in_=wkT_d.ap())
            nc.sync.dma_start(out=wv_sb, in_=wvT_d.ap())
            nc.sync.dma_start(out=pmat_sb, in_=pmat_d.ap())
            prefetch(1)
            phase1(0)
            # deferred bulk constant loads (needed only from attention onward)
            nc.sync.dma_start(out=tri_sb, in_=tri_d.ap())
            nc.sync.dma_start(out=wo_sb, in_=woT_d.ap())
            # depth-1 software pipeline: phase1(qc+1) is emitted right before
            # attention_body(qc), so late (ACT-bound) attentions still have
            # PE filler work; tails are deferred one body to keep their
            # serial psum chains off the PE queue's critical path
            tails = {}
            for qc in range(LC):
                if qc + 1 < LC:
                    phase1(qc + 1)
                    prefetch(qc + 2)
                tails[qc] = attention_body(qc)
                if qc - 1 in tails:
                    attention_tail(qc - 1, *tails.pop(qc - 1))
            attention_tail(LC - 1, *tails.pop(LC - 1), last=True)

    nc.compile()
    return nc


def _host_prep(x, wq, wk, wv, wo, L):
    """Build per-core input maps (numpy only)."""
    import ml_dtypes
    BF = ml_dtypes.bfloat16
    LC = L // 512
    HC = HIDDEN // 128

    x2 = x.reshape(L, HIDDEN)
    xT = np.ascontiguousarray(x2.T)                        # [1024, L]
    # pack [LC, 128, HC, 512]: [n, p, c, m] = xT[c*128+p, n*512+m]
    xt_pack = np.ascontiguousarray(
        xT.reshape(HC, 128, LC, 512).transpose(2, 1, 0, 3).astype(BF))

    # rope tables, transposed + duplicated for the two heads on each core
    inv_freq = 1.0 / (ROPE_BASE ** (np.arange(0, HEAD_DIM, 2, dtype=np.float64)
                                    / HEAD_DIM))
    freqs = np.arange(L, dtype=np.float64)[:, None] * inv_freq[None, :]
    emb = np.concatenate([freqs, freqs], axis=-1)          # [L, 64]
    cosT = np.cos(emb).T.astype(np.float32)                # [64, L]
    sinT = np.sin(emb).T.astype(np.float32)
    cosT2 = np.concatenate([cosT, cosT], axis=0)           # [128, L]
    sinT2 = np.concatenate([sinT, sinT], axis=0)
    # cs pack [LC, 128, 2, 512]
    cs_pack = np.ascontiguousarray(
        np.stack([cosT2.reshape(128, LC, 512), sinT2.reshape(128, LC, 512)],
                 axis=2).transpose(1, 0, 2, 3).astype(BF))

    # intra-block triangle mask [128, 2, 128] (duplicated over head slot)
    kv = np.arange(128)[:, None]
    qq = np.arange(128)[None, :]
    tri = (qq >= kv).astype(BF)                            # [128, 128]
    tri2 = np.ascontiguousarray(np.repeat(tri[:, None, :], 2, axis=1))

    # rotate-half permutation (as matmul lhsT), block-diag for 2 heads
    P = np.zeros((64, 64), np.float32)
    P[np.arange(32) + 32, np.arange(32)] = -1.0
    P[np.arange(32), np.arange(32) + 32] = 1.0
    pmat = np.zeros((128, 128), np.float32)
    pmat[0:64, 0:64] = P
    pmat[64:128, 64:128] = P
    pmat = pmat.astype(BF)

    # per-head selector rows for the denominator-broadcast matmul
    osel = np.zeros((1, 2, 128), np.float32)
    osel[0, 0, 0:64] = 1.0
    osel[0, 1, 64:128] = 1.0

    def pack_w(w, scale=1.0):
        # lhsT pack [128, HC, 128]: [p, k, m] = w_c[m, k*128+p]
        wt = np.ascontiguousarray(w.T)                     # [1024 hid, 128 out]
        return np.ascontiguousarray(
            (wt.reshape(HC, 128, 128).transpose(1, 0, 2) * scale).astype(BF))

    in_maps = []
    for c in range(N_CORES):
        rows = slice(c * 128, (c + 1) * 128)
        in_maps.append({
            "xt": xt_pack,
            "wqT": pack_w(wq[rows, :], np.float32(1.0 / 8.0)),
            "wkT": pack_w(wk[rows, :]),
            "wvT": pack_w(wv[rows, :]),
            "woT": np.ascontiguousarray(wo[:, rows].T.astype(BF)),
            "cs": cs_pack,
            "tri": tri2,
            "pmat": pmat,
            "osel": osel,
        })
    return in_maps


def _ensure_profile_hook():
    """The agent image's antenv lacks axon_hooks; recreate it from the boot
    package so trace=True can capture NTFF profiles."""
    import sys, types
    try:
        from antenv.axon_hooks import get_axon_ntff_profile_hook  # noqa: F401
        return
    except ImportError:
        pass
    try:
        from trn_agent_boot.trn_boot import _ntff_profile_via_ctypes
        hook = _ntff_profile_via_ctypes('/opt/axon/libaxon_pjrt.so')
    except Exception:
        hook = None
    mod = types.ModuleType("antenv.axon_hooks")
    mod.get_axon_ntff_profile_hook = lambda: hook
    mod.set_axon_ntff_profile_hook = lambda h: None
    sys.modules["antenv.axon_hooks"] = mod


def _run(x, wq, wk, wv, wo, trace=False, trace_cores=None):
    from concourse.bass_utils import run_bass_kernel_spmd

    if trace:
        _ensure_profile_hook()

    B, L, D = x.shape
    assert (B, D) == (1, HIDDEN)
    if L not in _CACHE:
        _CACHE[L] = _build(L)
    nc = _CACHE[L]
    in_maps = _host_prep(np.asarray(x, np.float32), wq, wk, wv, wo, L)
    res = run_bass_kernel_spmd(
        nc, in_maps, core_ids=list(range(N_CORES)),
        trace=trace, trace_cores=trace_cores)
    acc = np.zeros((HIDDEN, L), np.float64)
    for r in res.results:
        acc += r["yT"].astype(np.float64)
    y = np.ascontiguousarray(acc.T.astype(np.float32)).reshape(1, L, HIDDEN)
    return y, res


def kernel(x, wq, wk, wv, wo):
    y, _ = _run(np.asarray(x), np.asarray(wq), np.asarray(wk),
                np.asarray(wv), np.asarray(wo))
    return y
